# revision 1
# baseline (speedup 1.0000x reference)
"""Trainium2 Bass kernel for the Binary-MLP (nn_Binary0) problem.

Strategy (8-way batch-parallel, 1024 rows/core):
  fc1: h1 = x @ sign(w1).T        -- bf16x3 split of x (fp32-exact), bf16 +-1 weights
       a1 = sign(h1 - t1)          -- thresholds fold bias+BN (host fp64), fp8 out
  fc2: h2 = a1 @ sign(w2).T        -- fp8 DoubleRow (exact: +-1 products, fp32 psum)
       a2 = sign(h2 - t2)
  fc3: h3 = a2 @ sign(w3).T        -- fp8 DoubleRow
       h3c = clip(h3*s3 + c3, -1, 1)
  fc4: logits = h3c @ w4.T + b4    -- fp32, fused into fc3 loop (psum accumulate)
  out = log_softmax(logits)        -- on-chip, free-dim reduce

All activations live feature-major [feature, batch] so per-feature thresholds
are per-partition ACT bias vectors, and each layer's sign-output writes land
directly in the DoubleRow-paired [k, 2, batch] slab layout the next layer needs.
"""
import sys

for _p in ("/opt/trn_rl_repo",):
    if _p not in sys.path:
        sys.path.insert(0, _p)

import numpy as np
import ml_dtypes

import concourse.bass as bass
import concourse.tile as tile
import concourse.mybir as mybir
from concourse.bass_utils import run_bass_kernel_spmd

F32 = mybir.dt.float32
BF16 = mybir.dt.bfloat16
FP8 = mybir.dt.float8e4
NP_FP8 = mybir.dt.np(FP8)
NP_BF16 = ml_dtypes.bfloat16

EPS = 1e-5
NCORES = 8
B = 8192
BC = B // NCORES            # 1024 batch rows per core
D0, D1, D2 = 784, 3072, 6144
K1 = 7                      # fc1 contraction tiles (896 = 7*128, zero-padded)
D0P = K1 * 128
NPASS = 3                   # bf16 splits of x
NJ1 = D1 // 128             # 24 fc1 output feature tiles
NT2 = D1 // 256             # 12 fc2 DoubleRow contraction tiles
NJ2 = D2 // 128             # 48
NT3 = D2 // 256             # 24 fc3 DoubleRow contraction tiles
NJ3 = D2 // 128             # 48
JB = 4                      # j-tiles per streamed weight slab
NB = 2                      # 512-wide batch halves of BC
NBCH = BC // 128            # 8 batch chunks for fc4
NCLS = 16                   # padded class dim (10 real)

TRACE = False               # test.py sets True for profiling
TRACE_DIR = None
LAST_EXEC_NS = None

DR = mybir.MatmulPerfMode.DoubleRow
ACTF = mybir.ActivationFunctionType
ALU = mybir.AluOpType


def _legalize_multiwait(nc):
    """This container's walrus build rejects >1 sync-wait on one instruction
    (codegen 'Too many sync wait commands'); split extra waits into NoOps."""
    n = 0
    for f in nc.m.functions:
        for blk in f.blocks:
            insts = list(blk.instructions)
            new = []
            changed = False
            for ins in insts:
                si = ins.sync_info
                waits = list(si.on_wait) if (si is not None and si.on_wait) else []
                if len(waits) > 1:
                    for k, w in enumerate(waits[:-1]):
                        nop = mybir.InstNoOp(name=f"{ins.name}-sw{k}", ins=[], outs=[])
                        nop.engine = ins.engine
                        nop.sync_info = mybir.SyncInfo(on_wait=[w], on_update=[])
                        new.append(nop)
                        n += 1
                    ins.sync_info = mybir.SyncInfo(
                        on_wait=[waits[-1]], on_update=list(si.on_update or [])
                    )
                    changed = True
                new.append(ins)
            if changed:
                blk.instructions = new
    return n


def _build_nc():
    nc = bass.Bass("TRN2")

    x3t = nc.dram_tensor("x3t", [NPASS * K1, 128, BC], BF16, kind="ExternalInput")
    w1t = nc.dram_tensor("w1t", [K1, 128, D1], BF16, kind="ExternalInput")
    w2p = nc.dram_tensor("w2p", [NJ2 // JB, NT2, 128, 2, JB * 128], FP8,
                         kind="ExternalInput")
    w3p = nc.dram_tensor("w3p", [NJ3 // JB, NT3, 128, 2, JB * 128], FP8,
                         kind="ExternalInput")
    w4t = nc.dram_tensor("w4t", [NJ3, 128, NCLS], F32, kind="ExternalInput")
    negt1 = nc.dram_tensor("negt1", [D1, 1], F32, kind="ExternalInput")
    negt2 = nc.dram_tensor("negt2", [D2, 1], F32, kind="ExternalInput")
    s3d = nc.dram_tensor("s3d", [D2, 1], F32, kind="ExternalInput")
    c3d = nc.dram_tensor("c3d", [D2, 1], F32, kind="ExternalInput")
    b4bc = nc.dram_tensor("b4bc", [128, NCLS], F32, kind="ExternalInput")
    out = nc.dram_tensor("out", [BC, 10], F32, kind="ExternalOutput")

    with tile.TileContext(nc) as tc:
        with (
            tc.tile_pool(name="consts", bufs=1) as consts,
            tc.tile_pool(name="a1p", bufs=1) as a1p,
            tc.tile_pool(name="a2p", bufs=1) as a2p,
            tc.tile_pool(name="psum", bufs=4, space="PSUM") as psum,
            tc.tile_pool(name="psum_logit", bufs=1, space="PSUM") as psum_logit,
        ):
            # ---- constants ----
            nt1 = consts.tile([128, NJ1], F32)
            nc.sync.dma_start(out=nt1, in_=negt1.rearrange("(t p) o -> p (t o)", p=128))
            nt2 = consts.tile([128, NJ2], F32)
            nc.sync.dma_start(out=nt2, in_=negt2.rearrange("(t p) o -> p (t o)", p=128))
            s3s = consts.tile([128, NJ3], F32)
            nc.sync.dma_start(out=s3s, in_=s3d.rearrange("(t p) o -> p (t o)", p=128))
            c3s = consts.tile([128, NJ3], F32)
            nc.sync.dma_start(out=c3s, in_=c3d.rearrange("(t p) o -> p (t o)", p=128))
            b4s = consts.tile([128, NCLS], F32)
            nc.sync.dma_start(out=b4s, in_=b4bc[:, :])
            w4s = consts.tile([128, NJ3, NCLS], F32)
            for j in range(NJ3):
                nc.sync.dma_start(out=w4s[:, j, :], in_=w4t[j])

            a1 = a1p.tile([128, NT2, 2, BC], FP8)
            a2 = a2p.tile([128, NT3, 2, BC], FP8)

            lg = psum_logit.tile([128, NBCH, NCLS], F32)
            nc.vector.memset(lg, 0.0)

            # ---- fc1: bf16x3 exact fp32 matmul + sign threshold ----
            with tc.tile_pool(name="fc1res", bufs=1) as fc1res:
                x3 = fc1res.tile([128, NPASS * K1, BC], BF16)
                for i in range(NPASS * K1):
                    nc.sync.dma_start(out=x3[:, i, :], in_=x3t[i])
                w1 = fc1res.tile([128, K1, D1], BF16)
                for k in range(K1):
                    nc.sync.dma_start(out=w1[:, k, :], in_=w1t[k])

                for j in range(NJ1):
                    for n in range(NB):
                        ps = psum.tile([128, 512], F32, tag="ps")
                        idx = 0
                        for k in range(K1):
                            for p in range(NPASS):
                                nc.tensor.matmul(
                                    ps,
                                    lhsT=w1[:, k, j * 128:(j + 1) * 128],
                                    rhs=x3[:, p * K1 + k, n * 512:(n + 1) * 512],
                                    start=(idx == 0),
                                    stop=(idx == K1 * NPASS - 1),
                                )
                                idx += 1
                        nc.scalar.activation(
                            out=a1[:, j // 2, j % 2, n * 512:(n + 1) * 512],
                            in_=ps,
                            func=ACTF.Sign,
                            bias=nt1[:, j:j + 1],
                            scale=1.0,
                        )

            # ---- fc2: fp8 DoubleRow + sign threshold ----
            with tc.tile_pool(name="w2s", bufs=2) as w2s:
                for jb in range(NJ2 // JB):
                    wt = w2s.tile([128, NT2, 2, JB * 128], FP8, tag="w2t")
                    for t in range(NT2):
                        nc.sync.dma_start(out=wt[:, t], in_=w2p[jb, t])
                    for j in range(JB):
                        jj = jb * JB + j
                        for n in range(NB):
                            ps = psum.tile([128, 512], F32, tag="ps")
                            for t in range(NT2):
                                nc.tensor.matmul(
                                    ps,
                                    lhsT=wt[:, t, :, j * 128:(j + 1) * 128],
                                    rhs=a1[:, t, :, n * 512:(n + 1) * 512],
                                    start=(t == 0),
                                    stop=(t == NT2 - 1),
                                    perf_mode=DR,
                                )
                            nc.scalar.activation(
                                out=a2[:, jj // 2, jj % 2, n * 512:(n + 1) * 512],
                                in_=ps,
                                func=ACTF.Sign,
                                bias=nt2[:, jj:jj + 1],
                                scale=1.0,
                            )

            # ---- fc3 (fp8 DoubleRow) + bn3/hardtanh + fused fc4 ----
            with (
                tc.tile_pool(name="w3s", bufs=2) as w3s,
                tc.tile_pool(name="h3p", bufs=3) as h3p,
            ):
                for jb in range(NJ3 // JB):
                    wt = w3s.tile([128, NT3, 2, JB * 128], FP8, tag="w3t")
                    for t in range(NT3):
                        nc.sync.dma_start(out=wt[:, t], in_=w3p[jb, t])
                    for j in range(JB):
                        jj = jb * JB + j
                        h3 = h3p.tile([128, BC], F32, tag="h3")
                        for n in range(NB):
                            ps = psum.tile([128, 512], F32, tag="ps")
                            for t in range(NT3):
                                nc.tensor.matmul(
                                    ps,
                                    lhsT=wt[:, t, :, j * 128:(j + 1) * 128],
                                    rhs=a2[:, t, :, n * 512:(n + 1) * 512],
                                    start=(t == 0),
                                    stop=(t == NT3 - 1),
                                    perf_mode=DR,
                                )
                            tmp = h3p.tile([128, 512], F32, tag="bn3tmp")
                            nc.scalar.activation(
                                out=tmp,
                                in_=ps,
                                func=ACTF.Identity,
                                bias=c3s[:, jj:jj + 1],
                                scale=s3s[:, jj:jj + 1],
                            )
                            nc.vector.tensor_scalar(
                                out=h3[:, n * 512:(n + 1) * 512],
                                in0=tmp,
                                scalar1=-1.0,
                                scalar2=1.0,
                                op0=ALU.max,
                                op1=ALU.min,
                            )
                        # fused fc4: logits[b,c] += h3c[:,b128].T @ w4[:,c]
                        for b in range(NBCH):
                            nc.tensor.matmul(
                                lg[:, b, :],
                                lhsT=h3[:, b * 128:(b + 1) * 128],
                                rhs=w4s[:, jj, :],
                                start=False,
                                stop=(jj == NJ3 - 1),
                                skip_group_check=True,
                            )

            # ---- epilogue: +b4, log_softmax over the 10 real classes ----
            with tc.tile_pool(name="epi", bufs=2) as epi:
                for b in range(NBCH):
                    lsb = epi.tile([128, NCLS], F32, tag="lsb")
                    nc.vector.tensor_add(lsb, lg[:, b, :], b4s)
                    mx = epi.tile([128, 1], F32, tag="mx")
                    nc.vector.tensor_reduce(
                        out=mx, in_=lsb[:, 0:10], axis=mybir.AxisListType.X,
                        op=ALU.max,
                    )
                    sh = epi.tile([128, 10], F32, tag="sh")
                    nc.vector.tensor_scalar(
                        out=sh, in0=lsb[:, 0:10], scalar1=mx, scalar2=None,
                        op0=ALU.subtract,
                    )
                    ex = epi.tile([128, 10], F32, tag="ex")
                    nc.scalar.activation(out=ex, in_=sh, func=ACTF.Exp)
                    sm = epi.tile([128, 1], F32, tag="sm")
                    nc.vector.tensor_reduce(
                        out=sm, in_=ex, axis=mybir.AxisListType.X, op=ALU.add,
                    )
                    ln = epi.tile([128, 1], F32, tag="ln")
                    nc.scalar.activation(out=ln, in_=sm, func=ACTF.Ln)
                    res = epi.tile([128, 10], F32, tag="res")
                    nc.vector.tensor_scalar(
                        out=res, in0=sh, scalar1=ln, scalar2=None,
                        op0=ALU.subtract,
                    )
                    nc.sync.dma_start(out=out[b * 128:(b + 1) * 128, :], in_=res)

    _legalize_multiwait(nc)
    return nc


def _split3(x):
    """x (fp32) -> three bf16 arrays summing to x with <=2^-25 rel error."""
    x1 = x.astype(NP_BF16)
    r1 = (x - x1.astype(np.float32)).astype(np.float32)
    x2 = r1.astype(NP_BF16)
    r2 = (r1 - x2.astype(np.float32)).astype(np.float32)
    x3 = r2.astype(NP_BF16)
    return x1, x2, x3


def _prep_inputs(inputs):
    f64 = {k: np.asarray(v, np.float64) for k, v in inputs.items()}
    x = np.asarray(inputs["x"], np.float32)

    s1 = f64["g1"] / np.sqrt(f64["v1"] + EPS)
    t1 = f64["m1"] - f64["b1"] - f64["be1"] / s1
    s2 = f64["g2"] / np.sqrt(f64["v2"] + EPS)
    t2 = f64["m2"] - f64["b2"] - f64["be2"] / s2
    s3 = f64["g3"] / np.sqrt(f64["v3"] + EPS)
    c3 = (f64["b3"] - f64["m3"]) * s3 + f64["be3"]

    shared = {}
    shared["negt1"] = (-t1).astype(np.float32).reshape(D1, 1)
    shared["negt2"] = (-t2).astype(np.float32).reshape(D2, 1)
    shared["s3d"] = s3.astype(np.float32).reshape(D2, 1)
    shared["c3d"] = c3.astype(np.float32).reshape(D2, 1)

    b4p = np.zeros(NCLS, np.float32)
    b4p[:10] = np.asarray(inputs["b4"], np.float32)
    shared["b4bc"] = np.broadcast_to(b4p, (128, NCLS)).copy()

    # w1: sign, pad 784->896, [K1,128,D1] bf16 (transposed to [in,out])
    w1b = np.sign(np.asarray(inputs["w1"], np.float32)).astype(np.float32)  # [D1,D0]
    w1tp = np.zeros((D0P, D1), np.float32)
    w1tp[:D0] = w1b.T
    shared["w1t"] = np.ascontiguousarray(
        w1tp.reshape(K1, 128, D1).astype(NP_BF16))

    # w2/w3: sign -> DoubleRow pair layout [njb, nt, 128, 2, JB*128] fp8
    def pack_dr(w, njt_in, njb_out):
        # w: [out, in]; transpose -> [in, out]; in = njt_in*256 = nt*2*128
        wT = np.sign(np.asarray(w, np.float32)).T  # [in, out]
        nin, nout = wT.shape
        nt = nin // 256
        a = wT.reshape(nt, 2, 128, nout).transpose(0, 2, 1, 3)  # [nt,128,2,out]
        a = a.reshape(nt, 128, 2, njb_out, JB * 128).transpose(3, 0, 1, 2, 4)
        return np.ascontiguousarray(a.astype(NP_FP8))

    shared["w2p"] = pack_dr(inputs["w2"], NT2, NJ2 // JB)
    shared["w3p"] = pack_dr(inputs["w3"], NT3, NJ3 // JB)

    # w4: [10, D2] -> [NJ3, 128, NCLS] fp32, padded classes zero
    w4 = np.asarray(inputs["w4"], np.float32)
    w4tp = np.zeros((D2, NCLS), np.float32)
    w4tp[:, :10] = w4.T
    shared["w4t"] = np.ascontiguousarray(w4tp.reshape(NJ3, 128, NCLS))

    # x: pad 784->896, transpose, split into 3 bf16 passes, per-core shard
    xp = np.zeros((B, D0P), np.float32)
    xp[:, :D0] = x
    xT = np.ascontiguousarray(xp.T)  # [D0P, B]
    x1, x2, x3 = _split3(xT)
    per_core = []
    for c in range(NCORES):
        sl = slice(c * BC, (c + 1) * BC)
        parts = [xi[:, sl].reshape(K1, 128, BC) for xi in (x1, x2, x3)]
        x3t = np.ascontiguousarray(np.concatenate(parts, axis=0))  # [21,128,BC]
        m = dict(shared)
        m["x3t"] = x3t
        per_core.append(m)
    return per_core


_NC_CACHE = None


def kernel(**inputs):
    global _NC_CACHE, LAST_EXEC_NS
    if _NC_CACHE is None:
        _NC_CACHE = _build_nc()
    nc = _NC_CACHE
    in_maps = _prep_inputs(inputs)
    kwargs = {}
    if TRACE:
        _install_ntff_shim()
        kwargs = dict(trace=True, tmpdir=TRACE_DIR)
    res = run_bass_kernel_spmd(nc, in_maps, core_ids=list(range(NCORES)), **kwargs)
    LAST_EXEC_NS = res.exec_time_ns
    return np.concatenate([res.results[c]["out"] for c in range(NCORES)], axis=0)


def _install_ntff_shim():
    """antenv.axon_hooks shim so trace=True works under axon (profiling only)."""
    import contextlib
    import ctypes
    import types

    if "antenv.axon_hooks" in sys.modules:
        return
    try:
        lib = ctypes.CDLL("/opt/axon/libaxon_pjrt.so")
        lib.axon_start_nrt_profile.argtypes = [
            ctypes.POINTER(ctypes.c_int64), ctypes.c_size_t]
        lib.axon_start_nrt_profile.restype = ctypes.c_int64
        lib.axon_stop_nrt_profile.argtypes = [ctypes.c_char_p]
        lib.axon_stop_nrt_profile.restype = ctypes.c_int64
    except (OSError, AttributeError):
        return

    @contextlib.contextmanager
    def _hook(output_dir, device_ids):
        import jax
        jax.devices()
        if device_ids:
            ids = (ctypes.c_int64 * len(device_ids))(*device_ids)
            rc = lib.axon_start_nrt_profile(ids, len(device_ids))
        else:
            rc = lib.axon_start_nrt_profile(None, 0)
        if rc != 0:
            raise RuntimeError(f"axon_start_nrt_profile rc={rc}")
        try:
            yield
        finally:
            n = lib.axon_stop_nrt_profile(str(output_dir).encode())
            print(f"ntff: {n} profile file(s) -> {output_dir}", file=sys.stderr)

    mod = types.ModuleType("antenv.axon_hooks")
    mod.get_axon_ntff_profile_hook = lambda: _hook
    mod.set_axon_ntff_profile_hook = lambda h: None
    sys.modules["antenv.axon_hooks"] = mod


# revision 4
# speedup vs baseline: 1.0966x; 1.0966x over previous
"""Trainium2 Bass kernel for the Binary-MLP (nn_Binary0) problem.

Strategy (8-way batch-parallel, 1024 rows/core):
  fc1: h1 = x @ sign(w1).T        -- bf16x3 split of x (fp32-exact), bf16 +-1 weights
       a1 = sign(h1 - t1)          -- thresholds fold bias+BN (host fp64), fp8 out
  fc2: h2 = a1 @ sign(w2).T        -- fp8 DoubleRow (exact: +-1 products, fp32 psum)
       a2 = sign(h2 - t2)
  fc3: h3 = a2 @ sign(w3).T        -- fp8 DoubleRow
       h3c = clip(h3*s3 + c3, -1, 1)
  fc4: logits.T = w4 @ h3c         -- fp32r (tf32-class, ample for fc4), fused into
                                      fc3 loop, [cls, batch] psum accumulation
  out = log_softmax(logits)        -- PE-transpose to [batch, cls], free-dim reduce

All activations live feature-major [feature, batch] so per-feature thresholds
are per-partition ACT bias vectors, and each layer's sign-output writes land
directly in the DoubleRow-paired [k, 2, batch] slab layout the next layer needs.
"""
import sys

for _p in ("/opt/trn_rl_repo",):
    if _p not in sys.path:
        sys.path.insert(0, _p)

import numpy as np
import ml_dtypes

import concourse.bass as bass
import concourse.tile as tile
import concourse.mybir as mybir
from concourse.bass_utils import run_bass_kernel_spmd
from concourse.masks import make_identity

F32 = mybir.dt.float32
F32R = mybir.dt.float32r
BF16 = mybir.dt.bfloat16
FP8 = mybir.dt.float8e4
NP_FP8 = mybir.dt.np(FP8)
NP_BF16 = ml_dtypes.bfloat16

EPS = 1e-5
NCORES = 8
B = 8192
BC = B // NCORES            # 1024 batch rows per core
D0, D1, D2 = 784, 3072, 6144
K1 = 7                      # fc1 contraction tiles (896 = 7*128, zero-padded)
D0P = K1 * 128
NPASS = 3                   # bf16 splits of x
NJ1 = D1 // 128             # 24 fc1 output feature tiles
NT2 = D1 // 256             # 12 fc2 DoubleRow contraction tiles
NJ2 = D2 // 128             # 48
NT3 = D2 // 256             # 24 fc3 DoubleRow contraction tiles
NJ3 = D2 // 128             # 48
JB = 4                      # j-tiles per streamed weight slab
NB = 2                      # 512-wide batch halves of BC
NBCH = BC // 128            # 8 batch chunks
NCLS = 16                   # padded class dim (10 real)

TRACE = False               # test.py sets True for profiling
TRACE_DIR = None
LAST_EXEC_NS = None

DR = mybir.MatmulPerfMode.DoubleRow
ACTF = mybir.ActivationFunctionType
ALU = mybir.AluOpType


def _legalize_multiwait(nc):
    """This container's walrus build rejects >1 sync-wait on one instruction
    (codegen 'Too many sync wait commands'); split extra waits into NoOps."""
    n = 0
    for f in nc.m.functions:
        for blk in f.blocks:
            insts = list(blk.instructions)
            new = []
            changed = False
            for ins in insts:
                si = ins.sync_info
                waits = list(si.on_wait) if (si is not None and si.on_wait) else []
                if len(waits) > 1:
                    for k, w in enumerate(waits[:-1]):
                        nop = mybir.InstNoOp(name=f"{ins.name}-sw{k}", ins=[], outs=[])
                        nop.engine = ins.engine
                        nop.sync_info = mybir.SyncInfo(on_wait=[w], on_update=[])
                        new.append(nop)
                        n += 1
                    ins.sync_info = mybir.SyncInfo(
                        on_wait=[waits[-1]], on_update=list(si.on_update or [])
                    )
                    changed = True
                new.append(ins)
            if changed:
                blk.instructions = new
    return n


def _build_nc():
    nc = bass.Bass("TRN2")

    x3t = nc.dram_tensor("x3t", [NPASS * K1, 128, BC], BF16, kind="ExternalInput")
    w1t = nc.dram_tensor("w1t", [K1, 128, D1], BF16, kind="ExternalInput")
    w2p = nc.dram_tensor("w2p", [NJ2 // JB, NT2, 128, 2, JB * 128], FP8,
                         kind="ExternalInput")
    w3p = nc.dram_tensor("w3p", [NJ3 // JB, NT3, 128, 2, JB * 128], FP8,
                         kind="ExternalInput")
    w4t = nc.dram_tensor("w4t", [128, NJ3 * NCLS], F32R, kind="ExternalInput")
    # cvec columns: [0:24]=-t1, [24:72]=-t2, [72:120]=s3, [120:168]=c3
    cvec = nc.dram_tensor("cvec", [128, NJ1 + 3 * NJ3], F32, kind="ExternalInput")
    b4c = nc.dram_tensor("b4c", [NCLS, 1], F32, kind="ExternalInput")
    out = nc.dram_tensor("out", [BC, 10], F32, kind="ExternalOutput")

    with tile.TileContext(nc) as tc:
        with (
            tc.tile_pool(name="consts", bufs=1) as consts,
            tc.tile_pool(name="a1p", bufs=1) as a1p,
            tc.tile_pool(name="a2p", bufs=1) as a2p,
            tc.tile_pool(name="psum", bufs=4, space="PSUM") as psum,
            tc.tile_pool(name="psum_lg", bufs=2, space="PSUM") as psum_lg,
            tc.tile_pool(name="psum_tp", bufs=2, space="PSUM") as psum_tp,
        ):
            a1 = a1p.tile([128, NT2, 2, BC], FP8)
            a2 = a2p.tile([128, NT3, 2, BC], FP8)

            # fc4 logits accumulators [cls, batch-half], pre-zeroed, start=False
            lg = [psum_lg.tile([NCLS, 512], F32, tag="lg", name=f"lg{i}")
                  for i in range(NB)]

            # ---- fc1: bf16x3 exact fp32 matmul + sign threshold ----
            with tc.tile_pool(name="fc1res", bufs=1) as fc1res:
                x3 = fc1res.tile([128, NPASS * K1, BC], BF16)
                w1 = fc1res.tile([128, K1, D1], BF16)
                # k-interleaved issue order so PE can start at k=0 arrival
                for k in range(K1):
                    for p3 in range(3):
                        nc.sync.dma_start(
                            out=w1[:, k, p3 * 1024:(p3 + 1) * 1024],
                            in_=w1t[k][:, p3 * 1024:(p3 + 1) * 1024],
                        )
                    for p in range(NPASS):
                        nc.sync.dma_start(out=x3[:, p * K1 + k, :],
                                          in_=x3t[p * K1 + k])
                    if k == 0:
                        cv = consts.tile([128, NJ1 + 3 * NJ3], F32)
                        nc.sync.dma_start(out=cv, in_=cvec[:, :])
                        nt1 = cv[:, 0:NJ1]
                        nt2 = cv[:, NJ1:NJ1 + NJ3]
                        s3s = cv[:, NJ1 + NJ3:NJ1 + 2 * NJ3]
                        c3s = cv[:, NJ1 + 2 * NJ3:NJ1 + 3 * NJ3]
                        w4s = consts.tile([128, NJ3, NCLS], F32R)
                        nc.sync.dma_start(
                            out=w4s, in_=w4t.rearrange("p (j c) -> p j c", c=NCLS))
                        b4s = consts.tile([NCLS, 1], F32)
                        nc.sync.dma_start(out=b4s, in_=b4c[:, :])
                        for n in range(NB):
                            nc.vector.memset(lg[n], 0.0)

                for j in range(NJ1):
                    for n in range(NB):
                        ps = psum.tile([128, 512], F32, tag="ps")
                        idx = 0
                        for k in range(K1):
                            for p in range(NPASS):
                                nc.tensor.matmul(
                                    ps,
                                    lhsT=w1[:, k, j * 128:(j + 1) * 128],
                                    rhs=x3[:, p * K1 + k, n * 512:(n + 1) * 512],
                                    start=(idx == 0),
                                    stop=(idx == K1 * NPASS - 1),
                                )
                                idx += 1
                        nc.scalar.activation(
                            out=a1[:, j // 2, j % 2, n * 512:(n + 1) * 512],
                            in_=ps,
                            func=ACTF.Sign,
                            bias=nt1[:, j:j + 1],
                            scale=1.0,
                        )

            # ---- fc2: fp8 DoubleRow + sign threshold ----
            with tc.tile_pool(name="w2s", bufs=2) as w2s:
                for jb in range(NJ2 // JB):
                    wt = w2s.tile([128, NT2, 2, JB * 128], FP8, tag="w2t")
                    for tg in range(NT2 // 2):  # 6 DMAs x 2 k'-tiles
                        nc.sync.dma_start(
                            out=wt[:, 2 * tg:2 * tg + 2],
                            in_=w2p[jb, 2 * tg:2 * tg + 2].rearrange(
                                "t p i n -> p t i n"),
                        )
                    for j in range(JB):
                        jj = jb * JB + j
                        for n in range(NB):
                            ps = psum.tile([128, 512], F32, tag="ps")
                            for t in range(NT2):
                                nc.tensor.matmul(
                                    ps,
                                    lhsT=wt[:, t, :, j * 128:(j + 1) * 128],
                                    rhs=a1[:, t, :, n * 512:(n + 1) * 512],
                                    start=(t == 0),
                                    stop=(t == NT2 - 1),
                                    perf_mode=DR,
                                )
                            nc.scalar.activation(
                                out=a2[:, jj // 2, jj % 2, n * 512:(n + 1) * 512],
                                in_=ps,
                                func=ACTF.Sign,
                                bias=nt2[:, jj:jj + 1],
                                scale=1.0,
                            )

            # ---- fc3 (fp8 DoubleRow) + bn3/hardtanh + fused fc4 (fp32r) ----
            with (
                tc.tile_pool(name="w3s", bufs=2) as w3s,
                tc.tile_pool(name="h3p", bufs=3) as h3p,
            ):
                for jb in range(NJ3 // JB):
                    wt = w3s.tile([128, NT3, 2, JB * 128], FP8, tag="w3t")
                    for tg in range(NT3 // 3):  # 8 DMAs x 3 k'-tiles
                        nc.sync.dma_start(
                            out=wt[:, 3 * tg:3 * tg + 3],
                            in_=w3p[jb, 3 * tg:3 * tg + 3].rearrange(
                                "t p i n -> p t i n"),
                        )
                    for j in range(JB):
                        jj = jb * JB + j
                        h3 = h3p.tile([128, BC], F32R, tag="h3")
                        for n in range(NB):
                            ps = psum.tile([128, 512], F32, tag="ps")
                            for t in range(NT3):
                                nc.tensor.matmul(
                                    ps,
                                    lhsT=wt[:, t, :, j * 128:(j + 1) * 128],
                                    rhs=a2[:, t, :, n * 512:(n + 1) * 512],
                                    start=(t == 0),
                                    stop=(t == NT3 - 1),
                                    perf_mode=DR,
                                )
                            tmp = h3p.tile([128, 512], F32, tag="bn3tmp")
                            nc.scalar.activation(
                                out=tmp,
                                in_=ps,
                                func=ACTF.Identity,
                                bias=c3s[:, jj:jj + 1],
                                scale=s3s[:, jj:jj + 1],
                            )
                            nc.vector.tensor_scalar(
                                out=h3[:, n * 512:(n + 1) * 512],
                                in0=tmp,
                                scalar1=-1.0,
                                scalar2=1.0,
                                op0=ALU.max,
                                op1=ALU.min,
                            )
                            # fused fc4 (fp32r): lg[n][c, b] += w4[c,:] @ h3c[:, b]
                            nc.tensor.matmul(
                                lg[n],
                                lhsT=w4s[:, jj, :],
                                rhs=h3[:, n * 512:(n + 1) * 512],
                                start=False,
                                stop=(jj == NJ3 - 1),
                                skip_group_check=True,
                            )

            # ---- epilogue: +b4, transpose [cls,b]->[b,cls], log_softmax ----
            with tc.tile_pool(name="epi", bufs=2) as epi:
                ident = consts.tile([NCLS, NCLS], F32)
                make_identity(nc, ident)
                lsb = epi.tile([NCLS, BC], F32, tag="lsb")
                for n in range(NB):
                    nc.scalar.activation(
                        out=lsb[:, n * 512:(n + 1) * 512],
                        in_=lg[n],
                        func=ACTF.Identity,
                        bias=b4s[:, 0:1],
                        scale=1.0,
                    )
                for b in range(NBCH):
                    tp = psum_tp.tile([128, NCLS], F32, tag="tp")
                    nc.tensor.transpose(
                        tp, lsb[:, b * 128:(b + 1) * 128], ident)
                    mx = epi.tile([128, 1], F32, tag="mx")
                    nc.vector.tensor_reduce(
                        out=mx, in_=tp[:, 0:10], axis=mybir.AxisListType.X,
                        op=ALU.max,
                    )
                    sh = epi.tile([128, 10], F32, tag="sh")
                    nc.vector.tensor_scalar(
                        out=sh, in0=tp[:, 0:10], scalar1=mx, scalar2=None,
                        op0=ALU.subtract,
                    )
                    ex = epi.tile([128, 10], F32, tag="ex")
                    nc.scalar.activation(out=ex, in_=sh, func=ACTF.Exp)
                    sm = epi.tile([128, 1], F32, tag="sm")
                    nc.vector.tensor_reduce(
                        out=sm, in_=ex, axis=mybir.AxisListType.X, op=ALU.add,
                    )
                    ln = epi.tile([128, 1], F32, tag="ln")
                    nc.scalar.activation(out=ln, in_=sm, func=ACTF.Ln)
                    res = epi.tile([128, 10], F32, tag="res")
                    nc.vector.tensor_scalar(
                        out=res, in0=sh, scalar1=ln, scalar2=None,
                        op0=ALU.subtract,
                    )
                    nc.sync.dma_start(out=out[b * 128:(b + 1) * 128, :], in_=res)

    _legalize_multiwait(nc)
    return nc


def _split3(x):
    """x (fp32) -> three bf16 arrays summing to x with <=2^-25 rel error."""
    x1 = x.astype(NP_BF16)
    r1 = (x - x1.astype(np.float32)).astype(np.float32)
    x2 = r1.astype(NP_BF16)
    r2 = (r1 - x2.astype(np.float32)).astype(np.float32)
    x3 = r2.astype(NP_BF16)
    return x1, x2, x3


def _prep_inputs(inputs):
    f64 = {k: np.asarray(v, np.float64) for k, v in inputs.items()
           if k != "x"}
    x = np.asarray(inputs["x"], np.float32)

    s1 = f64["g1"] / np.sqrt(f64["v1"] + EPS)
    t1 = f64["m1"] - f64["b1"] - f64["be1"] / s1
    s2 = f64["g2"] / np.sqrt(f64["v2"] + EPS)
    t2 = f64["m2"] - f64["b2"] - f64["be2"] / s2
    s3 = f64["g3"] / np.sqrt(f64["v3"] + EPS)
    c3 = (f64["b3"] - f64["m3"]) * s3 + f64["be3"]

    shared = {}
    # cvec [128, 24+48*3]: per-feature consts arranged [partition, tile]
    cvec = np.zeros((128, NJ1 + 3 * NJ3), np.float32)
    cvec[:, 0:NJ1] = (-t1).astype(np.float32).reshape(NJ1, 128).T
    cvec[:, NJ1:NJ1 + NJ3] = (-t2).astype(np.float32).reshape(NJ3, 128).T
    cvec[:, NJ1 + NJ3:NJ1 + 2 * NJ3] = s3.astype(np.float32).reshape(NJ3, 128).T
    cvec[:, NJ1 + 2 * NJ3:] = c3.astype(np.float32).reshape(NJ3, 128).T
    shared["cvec"] = np.ascontiguousarray(cvec)

    b4p = np.zeros((NCLS, 1), np.float32)
    b4p[:10, 0] = np.asarray(inputs["b4"], np.float32)
    shared["b4c"] = b4p

    # w1: sign, pad 784->896, [K1,128,D1] bf16 (transposed to [in,out])
    w1b = np.sign(np.asarray(inputs["w1"], np.float32)).astype(np.float32)
    w1tp = np.zeros((D0P, D1), np.float32)
    w1tp[:D0] = w1b.T
    shared["w1t"] = np.ascontiguousarray(
        w1tp.reshape(K1, 128, D1).astype(NP_BF16))

    # w2/w3: sign -> DoubleRow pair layout [njb, nt, 128, 2, JB*128] fp8
    def pack_dr(w, njb_out):
        wT = np.sign(np.asarray(w, np.float32)).T  # [in, out]
        nin, nout = wT.shape
        nt = nin // 256
        a = wT.reshape(nt, 2, 128, nout).transpose(0, 2, 1, 3)  # [nt,128,2,out]
        a = a.reshape(nt, 128, 2, njb_out, JB * 128).transpose(3, 0, 1, 2, 4)
        return np.ascontiguousarray(a.astype(NP_FP8))

    shared["w2p"] = pack_dr(inputs["w2"], NJ2 // JB)
    shared["w3p"] = pack_dr(inputs["w3"], NJ3 // JB)

    # w4: [10, D2] -> [128, NJ3*NCLS]: element [k, j*16+c] = w4[c, j*128+k]
    w4 = np.asarray(inputs["w4"], np.float32)
    w4tp = np.zeros((D2, NCLS), np.float32)
    w4tp[:, :10] = w4.T
    shared["w4t"] = np.ascontiguousarray(
        w4tp.reshape(NJ3, 128, NCLS).transpose(1, 0, 2).reshape(128, NJ3 * NCLS))

    # x: pad 784->896, transpose, split into 3 bf16 passes, per-core shard
    xp = np.zeros((B, D0P), np.float32)
    xp[:, :D0] = x
    xT = np.ascontiguousarray(xp.T)  # [D0P, B]
    x1, x2, x3 = _split3(xT)
    per_core = []
    for c in range(NCORES):
        sl = slice(c * BC, (c + 1) * BC)
        parts = [xi[:, sl].reshape(K1, 128, BC) for xi in (x1, x2, x3)]
        x3t = np.ascontiguousarray(np.concatenate(parts, axis=0))  # [21,128,BC]
        m = dict(shared)
        m["x3t"] = x3t
        per_core.append(m)
    return per_core


_NC_CACHE = None


def kernel(**inputs):
    global _NC_CACHE, LAST_EXEC_NS
    if _NC_CACHE is None:
        _NC_CACHE = _build_nc()
    nc = _NC_CACHE
    in_maps = _prep_inputs(inputs)
    kwargs = {}
    if TRACE:
        _install_ntff_shim()
        kwargs = dict(trace=True, tmpdir=TRACE_DIR)
    res = run_bass_kernel_spmd(nc, in_maps, core_ids=list(range(NCORES)), **kwargs)
    LAST_EXEC_NS = res.exec_time_ns
    return np.concatenate([res.results[c]["out"] for c in range(NCORES)], axis=0)


def _install_ntff_shim():
    """antenv.axon_hooks shim so trace=True works under axon (profiling only)."""
    import contextlib
    import ctypes
    import types

    if "antenv.axon_hooks" in sys.modules:
        return
    try:
        lib = ctypes.CDLL("/opt/axon/libaxon_pjrt.so")
        lib.axon_start_nrt_profile.argtypes = [
            ctypes.POINTER(ctypes.c_int64), ctypes.c_size_t]
        lib.axon_start_nrt_profile.restype = ctypes.c_int64
        lib.axon_stop_nrt_profile.argtypes = [ctypes.c_char_p]
        lib.axon_stop_nrt_profile.restype = ctypes.c_int64
    except (OSError, AttributeError):
        return

    @contextlib.contextmanager
    def _hook(output_dir, device_ids):
        import jax
        jax.devices()
        if device_ids:
            ids = (ctypes.c_int64 * len(device_ids))(*device_ids)
            rc = lib.axon_start_nrt_profile(ids, len(device_ids))
        else:
            rc = lib.axon_start_nrt_profile(None, 0)
        if rc != 0:
            raise RuntimeError(f"axon_start_nrt_profile rc={rc}")
        try:
            yield
        finally:
            n = lib.axon_stop_nrt_profile(str(output_dir).encode())
            print(f"ntff: {n} profile file(s) -> {output_dir}", file=sys.stderr)

    mod = types.ModuleType("antenv.axon_hooks")
    mod.get_axon_ntff_profile_hook = lambda: _hook
    mod.set_axon_ntff_profile_hook = lambda h: None
    sys.modules["antenv.axon_hooks"] = mod
